# revision 10
# baseline (speedup 1.0000x reference)
"""CrossAttention (PVT-style SR attention) Trainium2 Bass kernel.

Problem (hardcoded shapes): B=4, C=320, W=H=64, heads=5, hd=64, SR=2.
  q = (query_flat @ q_w.T)                                  # (B, N=4096, 320)
  x_ = conv2x2_s2(x, sr_w) + sr_b  -> LN -> kv = x_ @ kv_w.T
  out = softmax(q k^T / 8) v  -> proj -> (B, 320, 64, 64)

Sharding: 8 cores = (batch b in 0..3) x (query half in 0..1). Each core
computes conv+LN+KV for its batch (duplicated across the half-pair; cheap)
and attention + proj for its 2048 queries.

On-chip layout is transposed throughout: activations live as [C, N] tiles
(channels on partitions), which makes every matmul a natural lhsT/rhs pair
and turns the final output into the natural (C, W*H) layout of the result.

All matmuls run in bf16 (1 cycle/row on PE, low power -> avoids the HAM
50%-duty throttle that fp32/f32r triggers). Inputs are cast to bf16 on the
host so DMA feeds matmul-ready tiles directly. Error ~6e-3 vs 2e-2 gate.

Schedule: the ACT engine's 80 exp tiles (~92us) are the critical resource,
so attention starts as early as possible — right after LN + the head-4
slivers of kT/qT and v[0..3] exist — and the remaining kv/qproj work plus
the output projection drain into the PE slack of ACT-bound attention steps
as "filler" units. The LN mean is computed free inside the conv matmul
via an extra stats row appended to the mi=2 weight chunk.

Softmax runs without max-subtraction (scores are O(1)); the denominator
comes free via an all-ones 65th column of v; its reciprocal uses the fast
approximate DVE op (must read SBUF, not PSUM).
"""

import numpy as np
import ml_dtypes

import concourse.bacc as bacc
import concourse.mybir as mybir
import concourse.tile as tile
from concourse.bass_utils import run_bass_kernel_spmd

fp32 = mybir.dt.float32
bf16 = mybir.dt.bfloat16
BF = ml_dtypes.bfloat16
AF = mybir.ActivationFunctionType
OP = mybir.AluOpType

B, C, W, H = 4, 320, 64, 64
HEADS, HD, SR = 5, 64, 2
N = W * H            # 4096 queries per batch
NQ = N // 2          # 2048 queries per core
NK = (W // SR) * (H // SR)  # 1024 kv positions
SCALE = HD ** -0.5   # 0.125
LN_EPS = 1e-5
CH = [(0, 128), (128, 128), (256, 64)]  # C=320 partition chunks
TAPS = [(0, 0), (0, 1), (1, 0), (1, 1)]
CT = C + 1           # conv tap block width (stats column appended)
N_WARMUP = 8         # PE warmup matmuls (DVFS ramp) while first DMAs land

_cache = {}


def _build():
    nc = bacc.Bacc("TRN2", target_bir_lowering=False)

    d_q = nc.dram_tensor("q_slice", [C, NQ], bf16, kind="ExternalInput")
    d_x = nc.dram_tensor("x_b", [C, N], bf16, kind="ExternalInput")
    d_qwT = nc.dram_tensor("qwT", [C, C], bf16, kind="ExternalInput")
    d_kvwT = nc.dram_tensor("kvwT", [C, 2 * C], bf16, kind="ExternalInput")
    d_convT = nc.dram_tensor("convT", [C, 4 * CT], bf16, kind="ExternalInput")
    d_projT = nc.dram_tensor("projT", [C, C], bf16, kind="ExternalInput")
    d_bias = nc.dram_tensor("bias_t", [128, 9], fp32, kind="ExternalInput")
    d_srbsum = nc.dram_tensor("srbsum", [1, 1], fp32, kind="ExternalInput")
    d_vb = nc.dram_tensor("vb_row", [1, C], bf16, kind="ExternalInput")
    d_out = nc.dram_tensor("out", [C, NQ], bf16, kind="ExternalOutput")

    with tile.TileContext(nc) as tc:
        with tc.tile_pool(name="persist", bufs=1) as PP:
            # ---- persistent small tensors ----
            bias_t = PP.tile([128, 9], fp32, tag="bias", name="bias")
            nc.sync.dma_start(bias_t[:], d_bias[:])
            srb_t = bias_t[:, 0:3]   # cols 0-2 sr_b
            kb_t = bias_t[:, 3:6]    # cols 3-5 kv bias (k part)
            pb_t = bias_t[:, 6:9]    # cols 6-8 proj bias
            srbsum_t = PP.tile([1, 1], fp32, tag="srbsum", name="srbsum")
            nc.sync.dma_start(srbsum_t[:], d_srbsum[:])

            eps_t = PP.tile([1, 1], fp32, tag="eps", name="eps")
            nc.vector.memset(eps_t[:], LN_EPS)
            scr_t = PP.tile([1, 1], fp32, tag="scr", name="scr")
            # warm the Sqrt activation table while ACT has nothing else to do
            nc.scalar.activation(scr_t[:], eps_t[:], AF.Sqrt)
            ones5 = PP.tile([128, 5], bf16, tag="ones5", name="ones5")
            nc.vector.memset(ones5[:], 1.0)
            ones_row = PP.tile([1, 128], bf16, tag="ones_row", name="ones_row")
            nc.vector.memset(ones_row[:], 1.0)
            inv_c = PP.tile([128, 1], bf16, tag="inv_c", name="inv_c")
            nc.vector.memset(inv_c[:], 1.0 / C)

            vb_r = PP.tile([1, C], bf16, tag="vb_r", name="vb_r")
            nc.sync.dma_start(vb_r[:], d_vb[:])

            # persistent activation tensors
            qT_r = [PP.tile([128, NQ], bf16, tag=f"qT{i}", name=f"qT{i}") for i in range(3)]
            kT_r = [PP.tile([128, NK], bf16, tag=f"kT{i}", name=f"kT{i}") for i in range(3)]
            v_r = [PP.tile([128, 5 * (HD + 1)], bf16, tag=f"v{i}", name=f"v{i}") for i in range(8)]

            # weights + inputs, DMA'd straight into matmul-ready bf16 tiles
            convT_r = [PP.tile([128, 4 * CT], bf16, tag=f"cw{i}", name=f"cw{i}") for i in range(3)]
            x_r = [PP.tile([128, N], bf16, tag=f"x{i}", name=f"x{i}") for i in range(3)]
            qwT_r = [PP.tile([128, C], bf16, tag=f"qw{i}", name=f"qw{i}") for i in range(3)]
            qf_r = [PP.tile([128, NQ], bf16, tag=f"qf{i}", name=f"qf{i}") for i in range(3)]
            kvwT_r = [PP.tile([128, 2 * C], bf16, tag=f"kvw{i}", name=f"kvw{i}") for i in range(3)]
            projT_r = [PP.tile([128, C], bf16, tag=f"pw{i}", name=f"pw{i}") for i in range(3)]

            # DMA priority order: conv weights + x first, then q-side (first
            # attention block), then kv/proj weights.
            for ki, (ko, ks) in enumerate(CH):
                nc.sync.dma_start(convT_r[ki][:ks], d_convT[ko:ko + ks, :])
                for hf in range(2):
                    hsl = slice(hf * (N // 2), (hf + 1) * (N // 2))
                    nc.sync.dma_start(x_r[ki][:ks, hsl], d_x[ko:ko + ks, hsl])
            for ki, (ko, ks) in enumerate(CH):
                nc.sync.dma_start(qwT_r[ki][:ks], d_qwT[ko:ko + ks, :])
                nc.sync.dma_start(qf_r[ki][:ks], d_q[ko:ko + ks, :])
            for ki, (ko, ks) in enumerate(CH):
                nc.sync.dma_start(kvwT_r[ki][:ks], d_kvwT[ko:ko + ks, :])
            for ki, (ko, ks) in enumerate(CH):
                nc.sync.dma_start(projT_r[ki][:ks], d_projT[ko:ko + ks, :])

            # ---------- phase 0: PE warmup (DVFS ramp during DMA wait) ------
            wz = PP.tile([128, 512], bf16, tag="wz", name="wz")
            nc.vector.memset(wz[:], 0.0)
            with tc.tile_pool(name="ps_w", bufs=1, space="PSUM") as PSW:
                wp = PSW.tile([128, 512], fp32, tag="wp", name="wp")
                for _ in range(N_WARMUP):
                    nc.tensor.matmul(wp[:], wz[:, :128], wz[:], start=True, stop=True)

            # ---------- phase 1: conv (with fused mean row) ----------
            LNP = tc.alloc_tile_pool(name="ln", bufs=1)  # spans conv->kv
            xconv_r = [LNP.tile([128, NK], bf16, tag=f"xc{i}", name=f"xc{i}") for i in range(3)]
            xsq_r = [LNP.tile([128, NK], bf16, tag=f"xq{i}", name=f"xq{i}") for i in range(3)]
            xhat_r = [LNP.tile([128, NK], bf16, tag=f"xh{i}", name=f"xh{i}") for i in range(3)]
            mu = LNP.tile([1, NK], fp32, tag="mu", name="mu")

            with tc.tile_pool(name="ps_c", bufs=1, space="PSUM") as PSC:
                pc = [
                    PSC.tile([128, NK], fp32, tag="pc0", name="pc0"),
                    PSC.tile([128, NK], fp32, tag="pc1", name="pc1"),
                    PSC.tile([65, NK], fp32, tag="pc2", name="pc2"),
                ]
                # output chunk widths incl. the stats row on mi=2
                MS = [(0, 128), (128, 128), (256, 65)]
                for ki, (ko, ks) in enumerate(CH):
                    for hf in range(2):
                        hsl = slice(hf * (N // 2), (hf + 1) * (N // 2))
                        xv = x_r[ki][:ks, hsl].rearrange("c (i j) -> c i j", i=W // 2)
                        for t, (di, dj) in enumerate(TAPS):
                            tap = xv[:, di::2, dj::2]  # [ks, 16, 32]
                            for mi, (mo, ms) in enumerate(MS):
                                lhsT = convT_r[ki][:ks, t * CT + mo:t * CT + mo + ms]
                                nc.tensor.matmul(
                                    pc[mi][:ms, hf * 512:(hf + 1) * 512],
                                    lhsT,
                                    tap,
                                    start=(ki == 0 and t == 0),
                                    stop=(ki == 2 and t == 3),
                                )
                # evacuate conv psum with +sr_b (DVE, bf16) per half + square
                for hf in range(2):
                    hsl = slice(hf * 512, (hf + 1) * 512)
                    for mi, (mo, ms) in enumerate(CH):
                        nc.vector.tensor_scalar_add(
                            xconv_r[mi][:ms, hsl], pc[mi][:ms, hsl],
                            srb_t[:ms, mi:mi + 1]
                        )
                        nc.vector.tensor_tensor(
                            xsq_r[mi][:ms, hsl], xconv_r[mi][:ms, hsl],
                            xconv_r[mi][:ms, hsl], OP.mult
                        )
                # mean row: conv stats row + sum(sr_b)/C
                nc.vector.tensor_scalar_add(mu[:], pc[2][64:65, :], srbsum_t[:1, :1])

            # ---------- LN row chain + early qproj (PE fills DVE latency) ---
            S2 = tc.alloc_tile_pool(name="s2", bufs=1)
            PSKV = tc.alloc_tile_pool(name="ps_kv", bufs=2, space="PSUM")
            PSS = tc.alloc_tile_pool(name="ps_s", bufs=1, space="PSUM")
            s_sq = PSS.tile([1, NK], fp32, tag="s_sq", name="s_sq")
            for h in range(2):
                for ki, (ko, ks) in enumerate(CH):
                    nc.tensor.matmul(
                        s_sq[:, h * 512:(h + 1) * 512],
                        inv_c[:ks],
                        xsq_r[ki][:ks, h * 512:(h + 1) * 512],
                        start=(ki == 0), stop=(ki == 2),
                    )

            musq = S2.tile([1, NK], fp32, tag="musq", name="musq")
            nc.vector.tensor_tensor(musq[:], mu[:], mu[:], OP.mult)
            var = S2.tile([1, NK], fp32, tag="var", name="var")
            nc.vector.scalar_tensor_tensor(
                var[:], s_sq[:], LN_EPS, musq[:], OP.add, OP.subtract
            )
            PSS.release()  # s_sq consumed; frees 2 PSUM banks for attention
            sd = S2.tile([1, NK], fp32, tag="sd", name="sd")
            nc.scalar.activation(sd[:], var[:], AF.Sqrt)
            rstd = S2.tile([1, NK], fp32, tag="rstd", name="rstd")
            nc.vector.reciprocal_approx_fast(rstd[:], sd[:])
            # nmr = -(mu * rstd); xhat = xconv * rstd + nmr
            nmr = S2.tile([1, NK], fp32, tag="nmr", name="nmr")
            nc.vector.scalar_tensor_tensor(
                nmr[:], mu[:], -1.0, rstd[:], OP.mult, OP.mult
            )
            # warm the Exp table before attention needs it
            nc.scalar.activation(scr_t[:], eps_t[:], AF.Exp)
            rstd_bc = S2.tile([128, NK], fp32, tag="rstd_bc", name="rstd_bc")
            nc.gpsimd.partition_broadcast(rstd_bc[:], rstd[:])
            nmr_bc = S2.tile([128, NK], fp32, tag="nmr_bc", name="nmr_bc")
            nc.gpsimd.partition_broadcast(nmr_bc[:], nmr[:])

            # qproj for head 4 (mi=2), nt 0/1 — PE work while DVE chain runs
            def qproj_unit(mi, nt):
                mo, ms = CH[mi]
                pq = PSKV.tile([128, 512], fp32, tag="pkv", name="pq")
                for ki, (ko, ks) in enumerate(CH):
                    nc.tensor.matmul(
                        pq[:ms],
                        qwT_r[ki][:ks, mo:mo + ms],
                        qf_r[ki][:ks, nt * 512:(nt + 1) * 512],
                        start=(ki == 0), stop=(ki == 2),
                    )
                nc.vector.tensor_copy(
                    qT_r[mi][:ms, nt * 512:(nt + 1) * 512], pq[:ms]
                )

            qproj_unit(2, 0)
            qproj_unit(2, 1)

            # xhat h0 then h1 (2 bf16-ish TT ops per chunk-half)
            xt_r = [S2.tile([128, NK], bf16, tag=f"xt{i}", name=f"xt{i}") for i in range(3)]

            def xhat_half(h):
                hsl = slice(h * 512, (h + 1) * 512)
                for ki, (ko, ks) in enumerate(CH):
                    nc.vector.tensor_tensor(
                        xt_r[ki][:ks, hsl], xconv_r[ki][:ks, hsl],
                        rstd_bc[:ks, hsl], OP.mult
                    )
                    nc.vector.tensor_tensor(
                        xhat_r[ki][:ks, hsl], xt_r[ki][:ks, hsl],
                        nmr_bc[:ks, hsl], OP.add
                    )

            xhat_half(0)

            # ---------- phase 2 units (emitted pre-attention or as fillers) -
            def kT_unit(h, mi):
                mo, ms = CH[mi]
                pk = PSKV.tile([128, 512], fp32, tag="pkv", name="pk")
                for ki, (ko, ks) in enumerate(CH):
                    nc.tensor.matmul(
                        pk[:ms],
                        kvwT_r[ki][:ks, mo:mo + ms],
                        xhat_r[ki][:ks, h * 512:(h + 1) * 512],
                        start=(ki == 0), stop=(ki == 2),
                    )
                nc.vector.tensor_scalar_add(
                    kT_r[mi][:ms, h * 512:(h + 1) * 512],
                    pk[:ms], kb_t[:ms, mi:mi + 1]
                )

            def v_unit(mc):
                pv = PSKV.tile([128, C + 1], fp32, tag="pkv", name="pv")
                for ki, (ko, ks) in enumerate(CH):
                    nc.tensor.matmul(
                        pv[:, :C],
                        xhat_r[ki][:ks, mc * 128:(mc + 1) * 128],
                        kvwT_r[ki][:ks, C:2 * C],
                        start=(ki == 0), stop=False,
                    )
                nc.tensor.matmul(  # rank-1 v bias
                    pv[:, :C], ones_row[:], vb_r[:], start=False, stop=True,
                )
                dst = v_r[mc][:].rearrange("p (h d) -> p h d", h=5)
                nc.vector.tensor_copy(
                    dst[:, :, :HD],
                    pv[:, :C].rearrange("p (h d) -> p h d", h=5),
                )
                nc.vector.tensor_copy(dst[:, :, HD:HD + 1], ones5[:, :, None])

            # pre-attention minimum: kT(h0, mi2) + v[0..3]
            kT_unit(0, 2)
            for mc in range(4):
                v_unit(mc)
            # xhat h1 (DVE) — needed by kT(h1,*) and v[4..7] fillers
            xhat_half(1)

            # ---------- phase 3: attention with filler interleave ----------
            OT_r = [PP.tile([128, NQ], bf16, tag=f"OT{i}", name=f"OT{i}") for i in range(3)]

            fillers = []
            fillers.append(lambda: kT_unit(1, 2))
            for mc in range(4, 8):
                fillers.append(lambda mc=mc: v_unit(mc))
            fillers.append(lambda: qproj_unit(0, 0))
            fillers.append(lambda: kT_unit(0, 0))
            fillers.append(lambda: kT_unit(1, 0))
            fillers.append(lambda: qproj_unit(1, 0))
            fillers.append(lambda: kT_unit(0, 1))
            fillers.append(lambda: kT_unit(1, 1))
            fillers.append(lambda: qproj_unit(0, 1))
            fillers.append(lambda: qproj_unit(1, 1))
            fillers.append(lambda: qproj_unit(2, 2))
            fillers.append(lambda: qproj_unit(2, 3))
            fillers.append(lambda: qproj_unit(0, 2))
            fillers.append(lambda: qproj_unit(1, 2))
            fillers.append(lambda: qproj_unit(0, 3))
            fillers.append(lambda: qproj_unit(1, 3))

            with (
                tc.tile_pool(name="s3", bufs=4) as S3,
                tc.tile_pool(name="s4", bufs=8) as S4,
                tc.tile_pool(name="ps_qk", bufs=2, space="PSUM") as PSA,
                tc.tile_pool(name="ps_o", bufs=1, space="PSUM") as PSO,
            ):
                proj_queue = []  # (nt, mi) groups still to emit

                def drain_one():
                    """Pop one filler (kv/qproj first, then proj groups)."""
                    if fillers:
                        fillers.pop(0)()
                        return True
                    if proj_queue:
                        nt, mi = proj_queue.pop(0)
                        mo, ms = CH[mi]
                        nsl = slice(nt * 512, (nt + 1) * 512)
                        py = PSKV.tile([128, 512], fp32, tag="pkv", name="py")
                        for ki, (ko, ks) in enumerate(CH):
                            nc.tensor.matmul(
                                py[:ms],
                                projT_r[ki][:ks, mo:mo + ms],
                                OT_r[ki][:ks, nsl],
                                start=(ki == 0), stop=(ki == 2),
                            )
                        yt = S3.tile([128, 512], bf16, tag="yt", name="yt")
                        nc.vector.tensor_scalar_add(
                            yt[:ms], py[:ms], pb_t[:ms, mi:mi + 1]
                        )
                        nc.sync.dma_start(d_out[mo:mo + ms, nsl], yt[:ms])
                        return True
                    return False

                def attn_block(cols, pops):
                    """cols: two (h, nt) column assignments for one ps tile.
                    pops: fillers to drain per mc step. AV lags QK by 2 steps
                    so exp never sits on the PE critical path."""
                    po = [
                        PSO.tile([HD + 1, 512], fp32, tag=f"po{i}", name=f"po{i}")
                        for i in range(2)
                    ]
                    pending = []

                    def do_av(ppt, pmc, last=False):
                        for i, (h, nt) in enumerate(cols):
                            vsl = slice(h * (HD + 1), (h + 1) * (HD + 1))
                            nc.tensor.matmul(
                                po[i][:], v_r[pmc][:, vsl],
                                ppt[:, i * 512:(i + 1) * 512],
                                start=(pmc == 0), stop=last,
                            )

                    for mc in range(8):
                        ps_s = PSA.tile([128, 1024], fp32, tag="ps", name="ps")
                        for i, (h, nt) in enumerate(cols):
                            ci, off = h // 2, (h % 2) * 64
                            nc.tensor.matmul(
                                ps_s[:, i * 512:(i + 1) * 512],
                                kT_r[ci][off:off + 64, mc * 128:(mc + 1) * 128],
                                qT_r[ci][off:off + 64, nt * 512:(nt + 1) * 512],
                                start=True, stop=True,
                            )
                        pt = S3.tile([128, 1024], bf16, tag="pt", name="pt")
                        nc.scalar.activation(pt[:], ps_s[:], AF.Exp, scale=SCALE)
                        pending.append((pt, mc))
                        if len(pending) > 2:
                            do_av(*pending.pop(0))
                        for _ in range(pops):
                            drain_one()
                    while pending:
                        ppt, pmc = pending.pop(0)
                        do_av(ppt, pmc, last=(pmc == 7))

                    # free po fast: write UNNORMALIZED rows + denom copy now;
                    # the reciprocal+broadcast+multiply runs later as a filler
                    # (must precede proj of this nt — FIFO queue guarantees it)
                    for i, (h, nt) in enumerate(cols):
                        ci, off = h // 2, (h % 2) * 64
                        nsl = slice(nt * 512, (nt + 1) * 512)
                        drow = S4.tile([1, 512], fp32, tag="drow", name="drow")
                        nc.vector.tensor_copy(drow[:], po[i][HD:HD + 1, :])
                        nc.vector.tensor_copy(
                            OT_r[ci][off:off + 64, nsl], po[i][:HD, :]
                        )

                        def norm_unit(ci=ci, off=off, nsl=nsl, drow=drow):
                            rrow = S3.tile([1, 512], fp32, tag="rrow", name="rrow")
                            nc.vector.reciprocal_approx_fast(rrow[:], drow[:])
                            # full-height broadcast so the in-place multiply's
                            # operands share a start partition (HW requirement)
                            rbc = S3.tile([128, 512], fp32, tag="rbc", name="rbc")
                            nc.gpsimd.partition_broadcast(rbc[:], rrow[:])
                            nc.vector.tensor_tensor(
                                OT_r[ci][off:off + 64, nsl],
                                OT_r[ci][off:off + 64, nsl],
                                rbc[off:off + 64, :], OP.mult,
                            )

                        fillers.append(norm_unit)

                # pops=1 while the early kv/qproj fillers drain; pops=2 in
                # the last blocks so the projection of late query tiles
                # overlaps attention instead of trailing serially (and at
                # throttled clock) after the final exp.
                bi = 0
                for nt2 in range(2):
                    nts = (2 * nt2, 2 * nt2 + 1)
                    attn_block([(4, nts[0]), (4, nts[1])], pops=2 if (bi < 2 or bi >= 6) else 1)
                    bi += 1
                    for nt in nts:
                        for pair in ((0, 1), (2, 3)):
                            attn_block(
                                [(pair[0], nt), (pair[1], nt)],
                                pops=2 if (bi < 2 or bi >= 6) else 1,
                            )
                            bi += 1
                        proj_queue.extend((nt, mi) for mi in range(3))
                while drain_one():
                    pass

            # close the manually-allocated pools (reverse order)
            PSKV.release()
            S2.release()
            LNP.release()

    nc.compile()
    return nc


def _prep_weights(q_w, kv_w, proj_w, proj_b, sr_w, sr_b, ln_g, ln_b):
    """Host-side weight preprocessing (fp32 math, bf16 on the wire)."""
    def pad_col(v):  # [320] -> [128, 3] column-major wrap
        out = np.zeros((128, 3), np.float32)
        out.reshape(-1, order="F")[:C] = v
        return out

    qwT = np.ascontiguousarray(q_w.T).astype(BF)
    kvw_g = kv_w * ln_g[None, :]
    kvwT = np.ascontiguousarray(kvw_g.T).astype(BF)  # [C, 2C]
    kvb = kv_w @ ln_b                                 # [2C]
    # conv tap blocks with the LN-mean stats column appended: [C, 4*(C+1)]
    blocks = []
    for (di, dj) in TAPS:
        blk = np.ascontiguousarray(sr_w[:, :, di, dj].T)      # [C(in), C(out)]
        ws = sr_w[:, :, di, dj].sum(0)[:, None] / C           # [C(in), 1]
        blocks.append(np.concatenate([blk, ws], axis=1))
    convT = np.concatenate(blocks, axis=1).astype(BF)
    projT = np.ascontiguousarray(proj_w.T).astype(BF)
    bias_t = np.concatenate(
        [pad_col(sr_b), pad_col(kvb[:C]), pad_col(proj_b)], axis=1
    )                                                 # [128, 9] fp32
    return {
        "qwT": qwT,
        "kvwT": kvwT,
        "convT": convT,
        "projT": projT,
        "bias_t": bias_t,
        "srbsum": np.array([[sr_b.sum() / C]], np.float32),
        "vb_row": np.ascontiguousarray(kvb[C:])[None, :].astype(BF),
    }


last_results = None


def kernel(query, x, q_w, kv_w, proj_w, proj_b, sr_w, sr_b, ln_g, ln_b):
    global last_results
    import os

    query = np.asarray(query, np.float32)
    x = np.asarray(x, np.float32)
    wmaps = _prep_weights(
        np.asarray(q_w, np.float32), np.asarray(kv_w, np.float32),
        np.asarray(proj_w, np.float32), np.asarray(proj_b, np.float32),
        np.asarray(sr_w, np.float32), np.asarray(sr_b, np.float32),
        np.asarray(ln_g, np.float32), np.asarray(ln_b, np.float32),
    )

    if "nc" not in _cache:
        _cache["nc"] = _build()
    nc = _cache["nc"]

    in_maps = []
    for core in range(8):
        b, half = core // 2, core % 2
        m = dict(wmaps)
        m["q_slice"] = np.ascontiguousarray(
            query[b, :, half * 32:(half + 1) * 32, :]
        ).reshape(C, NQ).astype(BF)
        m["x_b"] = np.ascontiguousarray(x[b]).reshape(C, N).astype(BF)
        in_maps.append(m)

    trace = os.environ.get("KERNEL_TRACE", "0") == "1"
    res = run_bass_kernel_spmd(
        nc, in_maps, core_ids=list(range(8)), trace=trace
    )
    last_results = res

    out = np.empty((B, C, W, H), np.float32)
    for core in range(8):
        b, half = core // 2, core % 2
        out[b, :, half * 32:(half + 1) * 32, :] = (
            res.results[core]["out"].astype(np.float32).reshape(C, 32, H)
        )
    return out


# revision 12
# speedup vs baseline: 1.0172x; 1.0172x over previous
"""CrossAttention (PVT-style SR attention) Trainium2 Bass kernel.

Problem (hardcoded shapes): B=4, C=320, W=H=64, heads=5, hd=64, SR=2.
  q = (query_flat @ q_w.T)                                  # (B, N=4096, 320)
  x_ = conv2x2_s2(x, sr_w) + sr_b  -> LN -> kv = x_ @ kv_w.T
  out = softmax(q k^T / 8) v  -> proj -> (B, 320, 64, 64)

Sharding: 8 cores = (batch b in 0..3) x (query half in 0..1). Each core
computes conv+LN+KV for its batch (duplicated across the half-pair; cheap)
and attention + proj for its 2048 queries.

On-chip layout is transposed throughout: activations live as [C, N] tiles
(channels on partitions), which makes every matmul a natural lhsT/rhs pair
and turns the final output into the natural (C, W*H) layout of the result.

All matmuls run in bf16 (1 cycle/row on PE, low power -> avoids the HAM
50%-duty throttle that fp32/f32r triggers). Inputs are cast to bf16 on the
host so DMA feeds matmul-ready tiles directly. Error ~6e-3 vs 2e-2 gate.

Schedule: the ACT engine's 80 exp tiles (~92us) are the critical resource,
so attention starts as early as possible — right after LN + the head-4
slivers of kT/qT and v[0..3] exist — and the remaining kv/qproj work plus
the output projection drain into the PE slack of ACT-bound attention steps
as "filler" units. The LN mean is computed free inside the conv matmul
via an extra stats row appended to the mi=2 weight chunk.

Softmax runs without max-subtraction (scores are O(1)); the denominator
comes free via an all-ones 65th column of v; its reciprocal uses the fast
approximate DVE op (must read SBUF, not PSUM).
"""

import numpy as np
import ml_dtypes

import concourse.bacc as bacc
import concourse.mybir as mybir
import concourse.tile as tile
from concourse.bass_utils import run_bass_kernel_spmd

fp32 = mybir.dt.float32
bf16 = mybir.dt.bfloat16
BF = ml_dtypes.bfloat16
AF = mybir.ActivationFunctionType
OP = mybir.AluOpType

B, C, W, H = 4, 320, 64, 64
HEADS, HD, SR = 5, 64, 2
N = W * H            # 4096 queries per batch
NQ = N // 2          # 2048 queries per core
NK = (W // SR) * (H // SR)  # 1024 kv positions
SCALE = HD ** -0.5   # 0.125
LN_EPS = 1e-5
CH = [(0, 128), (128, 128), (256, 64)]  # C=320 partition chunks
TAPS = [(0, 0), (0, 1), (1, 0), (1, 1)]
CT = C + 1           # conv tap block width (stats column appended)
N_WARMUP = 0         # PE warmup off: early duty feeds the HAM throttle trigger

_cache = {}


def _build():
    nc = bacc.Bacc("TRN2", target_bir_lowering=False)

    d_q = nc.dram_tensor("q_slice", [C, NQ], bf16, kind="ExternalInput")
    d_x = nc.dram_tensor("x_b", [C, N], bf16, kind="ExternalInput")
    d_qwT = nc.dram_tensor("qwT", [C, C], bf16, kind="ExternalInput")
    d_kvwT = nc.dram_tensor("kvwT", [C, 2 * C], bf16, kind="ExternalInput")
    d_convT = nc.dram_tensor("convT", [C, 4 * CT], bf16, kind="ExternalInput")
    d_projT = nc.dram_tensor("projT", [C, C], bf16, kind="ExternalInput")
    d_bias = nc.dram_tensor("bias_t", [128, 9], fp32, kind="ExternalInput")
    d_srbsum = nc.dram_tensor("srbsum", [1, 1], fp32, kind="ExternalInput")
    d_vb = nc.dram_tensor("vb_row", [1, C], bf16, kind="ExternalInput")
    d_out = nc.dram_tensor("out", [C, NQ], bf16, kind="ExternalOutput")

    with tile.TileContext(nc) as tc:
        with tc.tile_pool(name="persist", bufs=1) as PP:
            # ---- persistent small tensors ----
            bias_t = PP.tile([128, 9], fp32, tag="bias", name="bias")
            nc.sync.dma_start(bias_t[:], d_bias[:])
            srb_t = bias_t[:, 0:3]   # cols 0-2 sr_b
            kb_t = bias_t[:, 3:6]    # cols 3-5 kv bias (k part)
            pb_t = bias_t[:, 6:9]    # cols 6-8 proj bias
            srbsum_t = PP.tile([1, 1], fp32, tag="srbsum", name="srbsum")
            nc.sync.dma_start(srbsum_t[:], d_srbsum[:])

            eps_t = PP.tile([1, 1], fp32, tag="eps", name="eps")
            nc.vector.memset(eps_t[:], LN_EPS)
            scr_t = PP.tile([1, 1], fp32, tag="scr", name="scr")
            # warm the Sqrt activation table while ACT has nothing else to do
            nc.scalar.activation(scr_t[:], eps_t[:], AF.Sqrt)
            ones5 = PP.tile([128, 5], bf16, tag="ones5", name="ones5")
            nc.vector.memset(ones5[:], 1.0)
            ones_row = PP.tile([1, 128], bf16, tag="ones_row", name="ones_row")
            nc.vector.memset(ones_row[:], 1.0)
            inv_c = PP.tile([128, 1], bf16, tag="inv_c", name="inv_c")
            nc.vector.memset(inv_c[:], 1.0 / C)

            vb_r = PP.tile([1, C], bf16, tag="vb_r", name="vb_r")
            nc.sync.dma_start(vb_r[:], d_vb[:])

            # persistent activation tensors
            qT_r = [PP.tile([128, NQ], bf16, tag=f"qT{i}", name=f"qT{i}") for i in range(3)]
            kT_r = [PP.tile([128, NK], bf16, tag=f"kT{i}", name=f"kT{i}") for i in range(3)]
            v_r = [PP.tile([128, 5 * (HD + 1)], bf16, tag=f"v{i}", name=f"v{i}") for i in range(8)]

            # weights + inputs, DMA'd straight into matmul-ready bf16 tiles
            convT_r = [PP.tile([128, 4 * CT], bf16, tag=f"cw{i}", name=f"cw{i}") for i in range(3)]
            x_r = [PP.tile([128, N], bf16, tag=f"x{i}", name=f"x{i}") for i in range(3)]
            qwT_r = [PP.tile([128, C], bf16, tag=f"qw{i}", name=f"qw{i}") for i in range(3)]
            qf_r = [PP.tile([128, NQ], bf16, tag=f"qf{i}", name=f"qf{i}") for i in range(3)]
            kvwT_r = [PP.tile([128, 2 * C], bf16, tag=f"kvw{i}", name=f"kvw{i}") for i in range(3)]
            projT_r = [PP.tile([128, C], bf16, tag=f"pw{i}", name=f"pw{i}") for i in range(3)]

            # DMA priority order: conv weights + x first, then q-side (first
            # attention block), then kv/proj weights.
            for ki, (ko, ks) in enumerate(CH):
                nc.sync.dma_start(convT_r[ki][:ks], d_convT[ko:ko + ks, :])
                for hf in range(2):
                    hsl = slice(hf * (N // 2), (hf + 1) * (N // 2))
                    nc.sync.dma_start(x_r[ki][:ks, hsl], d_x[ko:ko + ks, hsl])
            for ki, (ko, ks) in enumerate(CH):
                nc.sync.dma_start(qwT_r[ki][:ks], d_qwT[ko:ko + ks, :])
                nc.sync.dma_start(qf_r[ki][:ks], d_q[ko:ko + ks, :])
            for ki, (ko, ks) in enumerate(CH):
                nc.sync.dma_start(kvwT_r[ki][:ks], d_kvwT[ko:ko + ks, :])
            for ki, (ko, ks) in enumerate(CH):
                nc.sync.dma_start(projT_r[ki][:ks], d_projT[ko:ko + ks, :])

            # ---------- phase 0: PE warmup (DVFS ramp during DMA wait) ------
            if N_WARMUP:
                wz = PP.tile([128, 512], bf16, tag="wz", name="wz")
                nc.vector.memset(wz[:], 0.0)
                with tc.tile_pool(name="ps_w", bufs=1, space="PSUM") as PSW:
                    wp = PSW.tile([128, 512], fp32, tag="wp", name="wp")
                    for _ in range(N_WARMUP):
                        nc.tensor.matmul(wp[:], wz[:, :128], wz[:], start=True, stop=True)

            # ---------- phase 1: conv (with fused mean row) ----------
            LNP = tc.alloc_tile_pool(name="ln", bufs=1)  # spans conv->kv
            xconv_r = [LNP.tile([128, NK], bf16, tag=f"xc{i}", name=f"xc{i}") for i in range(3)]
            xsq_r = [LNP.tile([128, NK], bf16, tag=f"xq{i}", name=f"xq{i}") for i in range(3)]
            xhat_r = [LNP.tile([128, NK], bf16, tag=f"xh{i}", name=f"xh{i}") for i in range(3)]
            mu = LNP.tile([1, NK], fp32, tag="mu", name="mu")

            with tc.tile_pool(name="ps_c", bufs=1, space="PSUM") as PSC:
                pc = [
                    PSC.tile([128, NK], fp32, tag="pc0", name="pc0"),
                    PSC.tile([128, NK], fp32, tag="pc1", name="pc1"),
                    PSC.tile([65, NK], fp32, tag="pc2", name="pc2"),
                ]
                # output chunk widths incl. the stats row on mi=2
                MS = [(0, 128), (128, 128), (256, 65)]
                for ki, (ko, ks) in enumerate(CH):
                    for hf in range(2):
                        hsl = slice(hf * (N // 2), (hf + 1) * (N // 2))
                        xv = x_r[ki][:ks, hsl].rearrange("c (i j) -> c i j", i=W // 2)
                        for t, (di, dj) in enumerate(TAPS):
                            tap = xv[:, di::2, dj::2]  # [ks, 16, 32]
                            for mi, (mo, ms) in enumerate(MS):
                                lhsT = convT_r[ki][:ks, t * CT + mo:t * CT + mo + ms]
                                nc.tensor.matmul(
                                    pc[mi][:ms, hf * 512:(hf + 1) * 512],
                                    lhsT,
                                    tap,
                                    start=(ki == 0 and t == 0),
                                    stop=(ki == 2 and t == 3),
                                )
                # evacuate conv psum with +sr_b (DVE, bf16) per half + square
                for hf in range(2):
                    hsl = slice(hf * 512, (hf + 1) * 512)
                    for mi, (mo, ms) in enumerate(CH):
                        nc.vector.tensor_scalar_add(
                            xconv_r[mi][:ms, hsl], pc[mi][:ms, hsl],
                            srb_t[:ms, mi:mi + 1]
                        )
                        nc.gpsimd.tensor_tensor(
                            xsq_r[mi][:ms, hsl], xconv_r[mi][:ms, hsl],
                            xconv_r[mi][:ms, hsl], OP.mult
                        )
                # mean row: conv stats row + sum(sr_b)/C
                nc.vector.tensor_scalar_add(mu[:], pc[2][64:65, :], srbsum_t[:1, :1])

            # ---------- LN row chain + early qproj (PE fills DVE latency) ---
            S2 = tc.alloc_tile_pool(name="s2", bufs=1)
            PSKV = tc.alloc_tile_pool(name="ps_kv", bufs=2, space="PSUM")
            PSS = tc.alloc_tile_pool(name="ps_s", bufs=1, space="PSUM")
            s_sq = PSS.tile([1, NK], fp32, tag="s_sq", name="s_sq")
            for h in range(2):
                for ki, (ko, ks) in enumerate(CH):
                    nc.tensor.matmul(
                        s_sq[:, h * 512:(h + 1) * 512],
                        inv_c[:ks],
                        xsq_r[ki][:ks, h * 512:(h + 1) * 512],
                        start=(ki == 0), stop=(ki == 2),
                    )

            musq = S2.tile([1, NK], fp32, tag="musq", name="musq")
            nc.vector.tensor_tensor(musq[:], mu[:], mu[:], OP.mult)
            var = S2.tile([1, NK], fp32, tag="var", name="var")
            nc.vector.scalar_tensor_tensor(
                var[:], s_sq[:], LN_EPS, musq[:], OP.add, OP.subtract
            )
            PSS.release()  # s_sq consumed; frees 2 PSUM banks for attention
            sd = S2.tile([1, NK], fp32, tag="sd", name="sd")
            nc.scalar.activation(sd[:], var[:], AF.Sqrt)
            rstd = S2.tile([1, NK], fp32, tag="rstd", name="rstd")
            nc.vector.reciprocal_approx_fast(rstd[:], sd[:])
            # nmr = -(mu * rstd); xhat = xconv * rstd + nmr
            nmr = S2.tile([1, NK], fp32, tag="nmr", name="nmr")
            nc.vector.scalar_tensor_tensor(
                nmr[:], mu[:], -1.0, rstd[:], OP.mult, OP.mult
            )
            # warm the Exp table before attention needs it
            nc.scalar.activation(scr_t[:], eps_t[:], AF.Exp)
            rstd_bc = S2.tile([128, NK], fp32, tag="rstd_bc", name="rstd_bc")
            nc.gpsimd.partition_broadcast(rstd_bc[:], rstd[:])
            nmr_bc = S2.tile([128, NK], fp32, tag="nmr_bc", name="nmr_bc")
            nc.gpsimd.partition_broadcast(nmr_bc[:], nmr[:])

            # qproj for head 4 (mi=2), nt 0/1 — PE work while DVE chain runs
            def qproj_unit(mi, nt):
                mo, ms = CH[mi]
                pq = PSKV.tile([128, 512], fp32, tag="pkv", name="pq")
                for ki, (ko, ks) in enumerate(CH):
                    nc.tensor.matmul(
                        pq[:ms],
                        qwT_r[ki][:ks, mo:mo + ms],
                        qf_r[ki][:ks, nt * 512:(nt + 1) * 512],
                        start=(ki == 0), stop=(ki == 2),
                    )
                nc.vector.tensor_copy(
                    qT_r[mi][:ms, nt * 512:(nt + 1) * 512], pq[:ms]
                )

            qproj_unit(2, 0)
            qproj_unit(2, 1)

            # xhat h0 then h1 (2 bf16-ish TT ops per chunk-half)
            xt_r = [S2.tile([128, NK], bf16, tag=f"xt{i}", name=f"xt{i}") for i in range(3)]

            def xhat_half(h):
                hsl = slice(h * 512, (h + 1) * 512)
                for ki, (ko, ks) in enumerate(CH):
                    nc.vector.tensor_tensor(
                        xt_r[ki][:ks, hsl], xconv_r[ki][:ks, hsl],
                        rstd_bc[:ks, hsl], OP.mult
                    )
                    nc.vector.tensor_tensor(
                        xhat_r[ki][:ks, hsl], xt_r[ki][:ks, hsl],
                        nmr_bc[:ks, hsl], OP.add
                    )

            xhat_half(0)

            # ---------- phase 2 units (emitted pre-attention or as fillers) -
            def kT_unit(h, mi):
                mo, ms = CH[mi]
                pk = PSKV.tile([128, 512], fp32, tag="pkv", name="pk")
                for ki, (ko, ks) in enumerate(CH):
                    nc.tensor.matmul(
                        pk[:ms],
                        kvwT_r[ki][:ks, mo:mo + ms],
                        xhat_r[ki][:ks, h * 512:(h + 1) * 512],
                        start=(ki == 0), stop=(ki == 2),
                    )
                nc.vector.tensor_scalar_add(
                    kT_r[mi][:ms, h * 512:(h + 1) * 512],
                    pk[:ms], kb_t[:ms, mi:mi + 1]
                )

            def v_unit(mc):
                pv = PSKV.tile([128, C + 1], fp32, tag="pkv", name="pv")
                for ki, (ko, ks) in enumerate(CH):
                    nc.tensor.matmul(
                        pv[:, :C],
                        xhat_r[ki][:ks, mc * 128:(mc + 1) * 128],
                        kvwT_r[ki][:ks, C:2 * C],
                        start=(ki == 0), stop=False,
                    )
                nc.tensor.matmul(  # rank-1 v bias
                    pv[:, :C], ones_row[:], vb_r[:], start=False, stop=True,
                )
                dst = v_r[mc][:].rearrange("p (h d) -> p h d", h=5)
                nc.vector.tensor_copy(
                    dst[:, :, :HD],
                    pv[:, :C].rearrange("p (h d) -> p h d", h=5),
                )
                nc.vector.tensor_copy(dst[:, :, HD:HD + 1], ones5[:, :, None])

            # pre-attention minimum: kT(h0, mi2) + v[0..3]
            kT_unit(0, 2)
            for mc in range(4):
                v_unit(mc)
            # xhat h1 (DVE) — needed by kT(h1,*) and v[4..7] fillers
            xhat_half(1)

            # ---------- phase 3: attention with filler interleave ----------
            OT_r = [PP.tile([128, NQ], bf16, tag=f"OT{i}", name=f"OT{i}") for i in range(3)]

            fillers = []
            fillers.append(lambda: kT_unit(1, 2))
            for mc in range(4, 8):
                fillers.append(lambda mc=mc: v_unit(mc))
            fillers.append(lambda: qproj_unit(0, 0))
            fillers.append(lambda: kT_unit(0, 0))
            fillers.append(lambda: kT_unit(1, 0))
            fillers.append(lambda: qproj_unit(1, 0))
            fillers.append(lambda: kT_unit(0, 1))
            fillers.append(lambda: kT_unit(1, 1))
            fillers.append(lambda: qproj_unit(0, 1))
            fillers.append(lambda: qproj_unit(1, 1))
            fillers.append(lambda: qproj_unit(2, 2))
            fillers.append(lambda: qproj_unit(2, 3))
            fillers.append(lambda: qproj_unit(0, 2))
            fillers.append(lambda: qproj_unit(1, 2))
            fillers.append(lambda: qproj_unit(0, 3))
            fillers.append(lambda: qproj_unit(1, 3))

            with (
                tc.tile_pool(name="s3", bufs=4) as S3,
                tc.tile_pool(name="s4", bufs=8) as S4,
                tc.tile_pool(name="ps_qk", bufs=2, space="PSUM") as PSA,
                tc.tile_pool(name="ps_o", bufs=1, space="PSUM") as PSO,
            ):
                proj_queue = []  # (nt, mi) groups still to emit

                def drain_one():
                    """Pop one filler (kv/qproj first, then proj groups)."""
                    if fillers:
                        fillers.pop(0)()
                        return True
                    if proj_queue:
                        nt, mi = proj_queue.pop(0)
                        mo, ms = CH[mi]
                        nsl = slice(nt * 512, (nt + 1) * 512)
                        py = PSKV.tile([128, 512], fp32, tag="pkv", name="py")
                        for ki, (ko, ks) in enumerate(CH):
                            nc.tensor.matmul(
                                py[:ms],
                                projT_r[ki][:ks, mo:mo + ms],
                                OT_r[ki][:ks, nsl],
                                start=(ki == 0), stop=(ki == 2),
                            )
                        yt = S3.tile([128, 512], bf16, tag="yt", name="yt")
                        nc.vector.tensor_scalar_add(
                            yt[:ms], py[:ms], pb_t[:ms, mi:mi + 1]
                        )
                        nc.sync.dma_start(d_out[mo:mo + ms, nsl], yt[:ms])
                        return True
                    return False

                def attn_block(cols, pops):
                    """cols: two (h, nt) column assignments for one ps tile.
                    pops: fillers to drain per mc step. AV lags QK by 2 steps
                    so exp never sits on the PE critical path."""
                    po = [
                        PSO.tile([HD + 1, 512], fp32, tag=f"po{i}", name=f"po{i}")
                        for i in range(2)
                    ]
                    pending = []

                    def do_av(ppt, pmc, last=False):
                        for i, (h, nt) in enumerate(cols):
                            vsl = slice(h * (HD + 1), (h + 1) * (HD + 1))
                            nc.tensor.matmul(
                                po[i][:], v_r[pmc][:, vsl],
                                ppt[:, i * 512:(i + 1) * 512],
                                start=(pmc == 0), stop=last,
                            )

                    for mc in range(8):
                        ps_s = PSA.tile([128, 1024], fp32, tag="ps", name="ps")
                        for i, (h, nt) in enumerate(cols):
                            ci, off = h // 2, (h % 2) * 64
                            nc.tensor.matmul(
                                ps_s[:, i * 512:(i + 1) * 512],
                                kT_r[ci][off:off + 64, mc * 128:(mc + 1) * 128],
                                qT_r[ci][off:off + 64, nt * 512:(nt + 1) * 512],
                                start=True, stop=True,
                            )
                        pt = S3.tile([128, 1024], bf16, tag="pt", name="pt")
                        nc.scalar.activation(pt[:], ps_s[:], AF.Exp, scale=SCALE)
                        pending.append((pt, mc))
                        if len(pending) > 2:
                            do_av(*pending.pop(0))
                        for _ in range(pops):
                            drain_one()
                    while pending:
                        ppt, pmc = pending.pop(0)
                        do_av(ppt, pmc, last=(pmc == 7))

                    # free po fast: write UNNORMALIZED rows + denom copy now;
                    # the reciprocal+broadcast+multiply runs later as a filler
                    # (must precede proj of this nt — FIFO queue guarantees it)
                    for i, (h, nt) in enumerate(cols):
                        ci, off = h // 2, (h % 2) * 64
                        nsl = slice(nt * 512, (nt + 1) * 512)
                        drow = S4.tile([1, 512], fp32, tag="drow", name="drow")
                        nc.vector.tensor_copy(drow[:], po[i][HD:HD + 1, :])
                        nc.vector.tensor_copy(
                            OT_r[ci][off:off + 64, nsl], po[i][:HD, :]
                        )

                        def norm_unit(ci=ci, off=off, nsl=nsl, drow=drow):
                            rrow = S3.tile([1, 512], fp32, tag="rrow", name="rrow")
                            nc.vector.reciprocal_approx_fast(rrow[:], drow[:])
                            # full-height broadcast so the in-place multiply's
                            # operands share a start partition (HW requirement)
                            rbc = S3.tile([128, 512], fp32, tag="rbc", name="rbc")
                            nc.gpsimd.partition_broadcast(rbc[:], rrow[:])
                            nc.vector.tensor_tensor(
                                OT_r[ci][off:off + 64, nsl],
                                OT_r[ci][off:off + 64, nsl],
                                rbc[off:off + 64, :], OP.mult,
                            )

                        fillers.append(norm_unit)

                bi = 0
                for nt2 in range(2):
                    nts = (2 * nt2, 2 * nt2 + 1)
                    attn_block([(4, nts[0]), (4, nts[1])], pops=1)
                    bi += 1
                    for nt in nts:
                        for pair in ((0, 1), (2, 3)):
                            attn_block([(pair[0], nt), (pair[1], nt)], pops=1)
                            bi += 1
                        proj_queue.extend((nt, mi) for mi in range(3))
                while drain_one():
                    pass

            # close the manually-allocated pools (reverse order)
            PSKV.release()
            S2.release()
            LNP.release()

    nc.compile()
    return nc


def _prep_weights(q_w, kv_w, proj_w, proj_b, sr_w, sr_b, ln_g, ln_b):
    """Host-side weight preprocessing (fp32 math, bf16 on the wire)."""
    def pad_col(v):  # [320] -> [128, 3] column-major wrap
        out = np.zeros((128, 3), np.float32)
        out.reshape(-1, order="F")[:C] = v
        return out

    qwT = np.ascontiguousarray(q_w.T).astype(BF)
    kvw_g = kv_w * ln_g[None, :]
    kvwT = np.ascontiguousarray(kvw_g.T).astype(BF)  # [C, 2C]
    kvb = kv_w @ ln_b                                 # [2C]
    # conv tap blocks with the LN-mean stats column appended: [C, 4*(C+1)]
    blocks = []
    for (di, dj) in TAPS:
        blk = np.ascontiguousarray(sr_w[:, :, di, dj].T)      # [C(in), C(out)]
        ws = sr_w[:, :, di, dj].sum(0)[:, None] / C           # [C(in), 1]
        blocks.append(np.concatenate([blk, ws], axis=1))
    convT = np.concatenate(blocks, axis=1).astype(BF)
    projT = np.ascontiguousarray(proj_w.T).astype(BF)
    bias_t = np.concatenate(
        [pad_col(sr_b), pad_col(kvb[:C]), pad_col(proj_b)], axis=1
    )                                                 # [128, 9] fp32
    return {
        "qwT": qwT,
        "kvwT": kvwT,
        "convT": convT,
        "projT": projT,
        "bias_t": bias_t,
        "srbsum": np.array([[sr_b.sum() / C]], np.float32),
        "vb_row": np.ascontiguousarray(kvb[C:])[None, :].astype(BF),
    }


last_results = None


def kernel(query, x, q_w, kv_w, proj_w, proj_b, sr_w, sr_b, ln_g, ln_b):
    global last_results
    import os

    query = np.asarray(query, np.float32)
    x = np.asarray(x, np.float32)
    wmaps = _prep_weights(
        np.asarray(q_w, np.float32), np.asarray(kv_w, np.float32),
        np.asarray(proj_w, np.float32), np.asarray(proj_b, np.float32),
        np.asarray(sr_w, np.float32), np.asarray(sr_b, np.float32),
        np.asarray(ln_g, np.float32), np.asarray(ln_b, np.float32),
    )

    if "nc" not in _cache:
        _cache["nc"] = _build()
    nc = _cache["nc"]

    in_maps = []
    for core in range(8):
        b, half = core // 2, core % 2
        m = dict(wmaps)
        m["q_slice"] = np.ascontiguousarray(
            query[b, :, half * 32:(half + 1) * 32, :]
        ).reshape(C, NQ).astype(BF)
        m["x_b"] = np.ascontiguousarray(x[b]).reshape(C, N).astype(BF)
        in_maps.append(m)

    trace = os.environ.get("KERNEL_TRACE", "0") == "1"
    res = run_bass_kernel_spmd(
        nc, in_maps, core_ids=list(range(8)), trace=trace
    )
    last_results = res

    out = np.empty((B, C, W, H), np.float32)
    for core in range(8):
        b, half = core // 2, core % 2
        out[b, :, half * 32:(half + 1) * 32, :] = (
            res.results[core]["out"].astype(np.float32).reshape(C, 32, H)
        )
    return out
